# revision 2
# baseline (speedup 1.0000x reference)
"""MCorr1d Trainium2 kernel (8 NeuronCores).

Problem (hardcoded from spec):
  in_    [1024, 64, 512]  fp32   (X, N, C_in)
  weight [16, 512, 512]   fp32   (KW, C_in, C_out)
  bias   [512]            fp32
  out    [64, 64, 512]    fp32   (Y, N, C_out)

  out[y, n, o] = bias[o] + sum_{w=0}^{15} sum_c in_[(y+1)*(w+1)-1, n, c] * weight[w, c, o]

Sharding: data-parallel over batch N: core i handles n in [8i, 8i+8).
Each core computes rows r = y*8 + n_local (512 rows) of out[:, n_slice, :].

Design (single-pass bf16, ~2.2e-3 max rel err):
  - Host packs, per core and tap, AW[w] = [C_IN, ROWS | C_OUT] bf16:
    the im2col-gathered A block (A[w,c,r] = in_[(y+1)(w+1)-1, n0+n, c])
    concatenated with weight[w] -- one contiguous 1MB DMA per tap.
  - 16 taps x 4 m-blocks x 4 k-chunks accumulating [128,128]x[128,512]
    matmuls into 4 PSUM banks (start on w=0, stop on w=15). 256 matmuls
    at the bf16 stream rate is the PE floor; the aw pool is 8 deep so
    DMA (16.8MB/core at ~370GB/s = 45us) always runs ahead of the PE.
  - Bias is added by the vector engine during the PSUM->SBUF drain
    (bias pre-broadcast to 128 partitions host-side), then DMA'd out.
  - PSUM/output pools are double-buffered so back-to-back invocations
    overlap across iterations of a timing loop.

Precision notes: bf16 single-pass gives ~2.2e-3 max rel err (gate 2e-2).
fp8 e4m3 DoubleRow would be ~1.4x faster on the PE but measures 3.6e-2
error -- over the gate -- and every hi/lo correction scheme costs >= 1.0
cycles/row, no faster than bf16, so bf16 is the right operating point.
"""

import contextlib

import numpy as np

X_LEN, N_BATCH, C_IN = 1024, 64, 512
KW, C_OUT = 16, 512
Y_OUT = 64
N_CORES = 8
N_PER = N_BATCH // N_CORES  # 8
ROWS = Y_OUT * N_PER  # 512
KC = C_IN // 128  # 4 contraction chunks
MC = ROWS // 128  # 4 row blocks
FD = ROWS + C_OUT  # 1024 combined A|W free dim

MODE = "bf16"

_XS = np.array([[(y + 1) * (w + 1) - 1 for y in range(Y_OUT)] for w in range(KW)])


def _build_nc(mode):
    return _build_nc_reps(mode, 1)


def _build_nc_reps(mode, reps, loop_n=0):
    import concourse.mybir as mybir
    import concourse.tile as tile
    from concourse import bacc

    f32 = mybir.dt.float32
    bf16 = mybir.dt.bfloat16

    nc = bacc.Bacc("TRN2", target_bir_lowering=False, debug=False,
                   num_devices=N_CORES)

    aw_t = nc.dram_tensor("aw", [KW, C_IN, FD], bf16,
                          kind="ExternalInput").ap()
    bias_t = nc.dram_tensor("bias", [128, C_OUT], f32,
                            kind="ExternalInput").ap()
    out_t = nc.dram_tensor("out", [ROWS, C_OUT], f32,
                           kind="ExternalOutput").ap()

    with tile.TileContext(nc) as tc:
        with tc.tile_pool(name="csb", bufs=1) as csb, \
             tc.tile_pool(name="awsb", bufs=8) as awsb, \
             tc.tile_pool(name="osb", bufs=2) as osb, \
             tc.tile_pool(name="ps", bufs=2, space="PSUM") as ps:

            bias_sb = csb.tile([128, C_OUT], f32, tag="bias")
            nc.sync.dma_start(bias_sb[:], bias_t[:])

            loop_cm = (tc.For_i(0, loop_n, 1) if loop_n
                       else contextlib.nullcontext())
            with loop_cm:
                for _rep in range(reps):
                    _emit_body(nc, awsb, osb, ps, bias_sb, aw_t, out_t, f32,
                               bf16)

    nc.compile()
    return nc


def _emit_body(nc, awsb, osb, ps, bias_sb, aw_t, out_t, f32, bf16):
    acc = [ps.tile([128, C_OUT], f32, name=f"acc{m}", tag=f"acc{m}")
           for m in range(MC)]

    for w in range(KW):
        t = awsb.tile([128, KC, FD], bf16, tag="aw", name="awt")
        nc.sync.dma_start(t[:], aw_t[w].rearrange("(k p) f -> p k f", p=128))
        last_w = (w == KW - 1)
        for m in range(MC):
            for k in range(KC):
                nc.tensor.matmul(
                    acc[m][:],
                    t[:, k, m * 128:(m + 1) * 128],
                    t[:, k, ROWS:FD],
                    start=(w == 0 and k == 0),
                    stop=(last_w and k == KC - 1))

    for m in range(MC):
        o_sb = osb.tile([128, C_OUT], f32, tag="o", name="o_sb")
        nc.vector.tensor_add(o_sb[:], acc[m][:], bias_sb[:])
        nc.sync.dma_start(out_t[m * 128:(m + 1) * 128, :], o_sb[:])


_NC_CACHE = {}


def _get_nc(mode):
    if mode not in _NC_CACHE:
        _NC_CACHE[mode] = _build_nc(mode)
    return _NC_CACHE[mode]


def _pack_inputs(in_, weight, bias, mode=None):
    """Host-side gather/pack. Returns list of per-core input maps."""
    import ml_dtypes

    in_ = np.asarray(in_, dtype=np.float32)
    weight = np.asarray(weight, dtype=np.float32)
    bias = np.asarray(bias, dtype=np.float32)

    # G[w, y, n, c] = in_[(y+1)(w+1)-1, n, c] -> A_all[w, c, y, n]
    G = in_[_XS.reshape(-1)].reshape(KW, Y_OUT, N_BATCH, C_IN)
    A_all = G.transpose(0, 3, 1, 2).astype(ml_dtypes.bfloat16)
    w_b = weight.astype(ml_dtypes.bfloat16)
    bias2 = np.ascontiguousarray(
        np.broadcast_to(bias.reshape(1, C_OUT), (128, C_OUT)))

    in_maps = []
    for c in range(N_CORES):
        n0 = c * N_PER
        aw = np.empty((KW, C_IN, FD), ml_dtypes.bfloat16)
        aw[:, :, :ROWS] = A_all[:, :, :, n0:n0 + N_PER].reshape(
            KW, C_IN, ROWS)
        aw[:, :, ROWS:] = w_b
        in_maps.append({"aw": aw, "bias": bias2})
    return in_maps


def kernel(in_, weight, bias):
    from concourse.bass_utils import run_bass_kernel_spmd

    nc = _get_nc(MODE)
    in_maps = _pack_inputs(in_, weight, bias, MODE)
    res = run_bass_kernel_spmd(nc, in_maps, core_ids=list(range(N_CORES)))
    # Each core returns out [ROWS, C_OUT] with rows r = y*N_PER + n_local.
    parts = [r["out"].reshape(Y_OUT, N_PER, C_OUT) for r in res.results]
    return np.concatenate(parts, axis=1).astype(np.float32)


# revision 3
# speedup vs baseline: 1.0969x; 1.0969x over previous
"""MCorr1d Trainium2 kernel (8 NeuronCores).

Problem (hardcoded from spec):
  in_    [1024, 64, 512]  fp32   (X, N, C_in)
  weight [16, 512, 512]   fp32   (KW, C_in, C_out)
  bias   [512]            fp32
  out    [64, 64, 512]    fp32   (Y, N, C_out)

  out[y, n, o] = bias[o] + sum_{w=0}^{15} sum_c in_[(y+1)*(w+1)-1, n, c] * weight[w, c, o]

Sharding: data-parallel over batch N: core i handles n in [8i, 8i+8).
Each core computes rows r = y*8 + n_local (512 rows) of out[:, n_slice, :].

Design (mixed bf16 + partial fp8-DoubleRow, ~1.51e-2 max rel err vs 2e-2 gate):
  - Taps 0-13 run single-pass bf16: host packs per tap AW[w] =
    [C_IN, ROWS | C_OUT] bf16 (im2col-gathered A block | weights), one
    contiguous 1MB DMA per tap; 14 taps x 4 m-blocks x 4 k-chunks of
    [128,128]x[128,512] matmuls accumulate into 4 PSUM banks.
  - Taps 14-15 run fp8 e4m3 with perf_mode=DoubleRow (2 MACs/cell/cycle):
    contraction pairs are packed [2j, 128p, 2i, f] with k = j*256+i*128+p.
    BALANCED scaling (W x16, A x1/16) keeps the product scale at 1 so these
    matmuls accumulate directly into the same PSUM groups as the bf16 taps
    -- no separate accumulator or rescale pass. Error is dominated by the
    fp8 taps: deterministic 1.51e-2 on the seeded inputs (HW == host sim).
  - Bias is added by the vector engine during the PSUM->SBUF drain (bias
    pre-broadcast to 128 partitions host-side), then DMA'd out.
  - PSUM/output pools are double-buffered; the aw pool is 8 deep so DMA
    (15.7MB/core at ~370GB/s) always runs ahead of the PE.

Measured floor notes: sustained PE column-feed rate is ~0.5ns/column
(clock/power wall) regardless of tiling; DoubleRow halves the fed columns
for its taps, which is where the gain comes from. Full fp8 would be ~2x
faster still but measures 3.6e-2 error -- over the gate.
"""

import contextlib

import numpy as np

X_LEN, N_BATCH, C_IN = 1024, 64, 512
KW, C_OUT = 16, 512
Y_OUT = 64
N_CORES = 8
N_PER = N_BATCH // N_CORES  # 8
ROWS = Y_OUT * N_PER  # 512
KC = C_IN // 128  # 4 contraction chunks
MC = ROWS // 128  # 4 row blocks
FD = ROWS + C_OUT  # 1024 combined A|W free dim

FP8_TAPS = 2  # last taps run fp8-e4m3 DoubleRow
FP8_SCALE = 16.0  # W x scale, A x 1/scale: product scale stays 1

MODE = "bf16"

_XS = np.array([[(y + 1) * (w + 1) - 1 for y in range(Y_OUT)] for w in range(KW)])


def _build_nc(mode):
    return _build_nc_reps(mode, 1)


def _build_nc_reps(mode, reps, loop_n=0):
    import concourse.mybir as mybir
    import concourse.tile as tile
    from concourse import bacc

    f32 = mybir.dt.float32
    bf16 = mybir.dt.bfloat16
    fp8 = mybir.dt.float8e4
    nbf = KW - FP8_TAPS

    nc = bacc.Bacc("TRN2", target_bir_lowering=False, debug=False,
                   num_devices=N_CORES)

    aw_t = nc.dram_tensor("aw", [nbf, C_IN, FD], bf16,
                          kind="ExternalInput").ap()
    aw8_t = nc.dram_tensor("aw8", [FP8_TAPS, 2, 128, 2, FD], fp8,
                           kind="ExternalInput").ap()
    bias_t = nc.dram_tensor("bias", [128, C_OUT], f32,
                            kind="ExternalInput").ap()
    out_t = nc.dram_tensor("out", [ROWS, C_OUT], f32,
                           kind="ExternalOutput").ap()

    with tile.TileContext(nc) as tc:
        with tc.tile_pool(name="csb", bufs=1) as csb, \
             tc.tile_pool(name="awsb", bufs=8) as awsb, \
             tc.tile_pool(name="osb", bufs=2) as osb, \
             tc.tile_pool(name="ps", bufs=2, space="PSUM") as ps:

            bias_sb = csb.tile([128, C_OUT], f32, tag="bias")
            nc.sync.dma_start(bias_sb[:], bias_t[:])

            loop_cm = (tc.For_i(0, loop_n, 1) if loop_n
                       else contextlib.nullcontext())
            with loop_cm:
                for _rep in range(reps):
                    _emit_body(nc, mybir, awsb, osb, ps, bias_sb, aw_t,
                               aw8_t, out_t, f32, bf16, fp8)

    nc.compile()
    return nc


def _emit_body(nc, mybir, awsb, osb, ps, bias_sb, aw_t, aw8_t, out_t, f32,
               bf16, fp8):
    nbf = KW - FP8_TAPS
    acc = [ps.tile([128, C_OUT], f32, name=f"acc{m}", tag=f"acc{m}")
           for m in range(MC)]

    for w in range(nbf):
        t = awsb.tile([128, KC, FD], bf16, tag="aw", name="awt")
        nc.sync.dma_start(t[:], aw_t[w].rearrange("(k p) f -> p k f", p=128))
        for m in range(MC):
            for k in range(KC):
                nc.tensor.matmul(
                    acc[m][:],
                    t[:, k, m * 128:(m + 1) * 128],
                    t[:, k, ROWS:FD],
                    start=(w == 0 and k == 0),
                    stop=False)

    for tp in range(FP8_TAPS):
        t8 = awsb.tile([128, 2, 2, FD], fp8, tag="aw8", name="aw8t")
        nc.sync.dma_start(t8[:], aw8_t[tp].rearrange("j p i f -> p j i f"))
        for j in range(2):
            for m in range(MC):
                nc.tensor.matmul(
                    acc[m][:],
                    t8[:, j, :, m * 128:(m + 1) * 128],
                    t8[:, j, :, ROWS:FD],
                    start=False,
                    stop=(tp == FP8_TAPS - 1 and j == 1),
                    perf_mode=mybir.MatmulPerfMode.DoubleRow)

    for m in range(MC):
        o_sb = osb.tile([128, C_OUT], f32, tag="o", name="o_sb")
        nc.vector.tensor_add(o_sb[:], acc[m][:], bias_sb[:])
        nc.sync.dma_start(out_t[m * 128:(m + 1) * 128, :], o_sb[:])


_NC_CACHE = {}


def _get_nc(mode):
    if mode not in _NC_CACHE:
        _NC_CACHE[mode] = _build_nc(mode)
    return _NC_CACHE[mode]


def _pack_inputs(in_, weight, bias, mode=None):
    """Host-side gather/pack. Returns list of per-core input maps."""
    import ml_dtypes

    in_ = np.asarray(in_, dtype=np.float32)
    weight = np.asarray(weight, dtype=np.float32)
    bias = np.asarray(bias, dtype=np.float32)
    nbf = KW - FP8_TAPS

    # G[w, y, n, c] = in_[(y+1)(w+1)-1, n, c] -> A_f32[w, c, y, n]
    G = in_[_XS.reshape(-1)].reshape(KW, Y_OUT, N_BATCH, C_IN)
    A_f32 = G.transpose(0, 3, 1, 2)
    A_bf = A_f32[:nbf].astype(ml_dtypes.bfloat16)
    w_b = weight[:nbf].astype(ml_dtypes.bfloat16)
    bias2 = np.ascontiguousarray(
        np.broadcast_to(bias.reshape(1, C_OUT), (128, C_OUT)))

    def to_dr(x):  # [C_IN, F] fp32 -> [2j,128p,2i,F] fp8 (k = j*256+i*128+p)
        return np.ascontiguousarray(
            x.reshape(2, 2, 128, -1).transpose(0, 2, 1, 3)
        ).astype(ml_dtypes.float8_e4m3fn)

    w8 = [to_dr(weight[nbf + t] * FP8_SCALE) for t in range(FP8_TAPS)]

    in_maps = []
    for c in range(N_CORES):
        n0 = c * N_PER
        aw = np.empty((nbf, C_IN, FD), ml_dtypes.bfloat16)
        aw[:, :, :ROWS] = A_bf[:, :, :, n0:n0 + N_PER].reshape(
            nbf, C_IN, ROWS)
        aw[:, :, ROWS:] = w_b
        aw8 = np.empty((FP8_TAPS, 2, 128, 2, FD), ml_dtypes.float8_e4m3fn)
        for t in range(FP8_TAPS):
            a_t = A_f32[nbf + t, :, :, n0:n0 + N_PER].reshape(
                C_IN, ROWS) * (1.0 / FP8_SCALE)
            aw8[t, :, :, :, :ROWS] = to_dr(a_t)
            aw8[t, :, :, :, ROWS:] = w8[t]
        in_maps.append({"aw": aw, "aw8": aw8, "bias": bias2})
    return in_maps


def kernel(in_, weight, bias):
    from concourse.bass_utils import run_bass_kernel_spmd

    nc = _get_nc(MODE)
    in_maps = _pack_inputs(in_, weight, bias, MODE)
    res = run_bass_kernel_spmd(nc, in_maps, core_ids=list(range(N_CORES)))
    # Each core returns out [ROWS, C_OUT] with rows r = y*N_PER + n_local.
    parts = [r["out"].reshape(Y_OUT, N_PER, C_OUT) for r in res.results]
    return np.concatenate(parts, axis=1).astype(np.float32)
